# revision 23
# baseline (speedup 1.0000x reference)
"""Causal multi-head attention (b=2, n=2048, d=768, 12 heads) on 8 TRN2 NeuronCores.

Sharding: batch x head-group. Core c handles batch c//4 and heads 3*(c%4) .. 3*(c%4)+2.
Each core gets xT = x[b].T plus W.T column slices for its 3 heads, computes the
unnormalized attention output (transposed) plus softmax denominators; the host
divides, transposes, and concatenates slabs into the full [2, 2048, 768].

Per-core algorithm (everything transposed so softmax reductions ride on matmuls):
  qT/kT/vT = (W.T slice).T @ xT            TensorE, per 512-col span
  v_nat[j, m] = transpose(vT) + ones column -> stationary [128, 65] per j-tile
  per head, per 512-col i-span:
    sT[j, i] = kT_h[:, jtile].T @ qT[:, span]   (psum, causally skipped/sliced)
    p = exp(sT)  unshifted (max causal score ~66 fits fp32); diagonal 128-blocks
        multiplied by a 0/1 triangular mask
    av[0:65, span] += v_nat[jtile].T @ p    (row 64 accumulates sum(p) = denom)
  av -> DRAM; host computes (av[0:64]/av[64]).T per head.

Perf-critical TRN2 facts baked in (measured on hardware):
  - every matmul keeps contraction K=128 (zero-padded kT/qT2): K<128 streams leave
    the PE HAM clock gate at 1.2 GHz and stop back-to-back pipelining (~3x slower)
  - float32r inputs: 1.06 cyc/row pipelined, ~12-bit mantissa
  - f32r tiles can only be produced by DVE/ACT compute ops (not memset/HWDGE-DMA),
    so zero-fills of f32r tiles are DVE copies from an f32 zeros tile
"""
import sys

if "/opt/trn_rl_repo" not in sys.path:
    sys.path.insert(0, "/opt/trn_rl_repo")

from contextlib import ExitStack

import numpy as np

import concourse.bass as bass
import concourse.tile as tile
from concourse import bacc, mybir, bass_utils
from concourse.masks import make_identity, make_upper_triangular

F32 = mybir.dt.float32
F32R = mybir.dt.float32r

P = 128
SPAN = 512
HD = 64

B, N, D, NH = 2, 2048, 768, 12
HL = 3                       # heads per core
DL = HL * HD                 # 192
N_CORES = 8
KT = D // P                  # 6 contraction chunks
NS = N // SPAN               # 4 spans
NT = N // P                  # 16 j-tiles
CPS = SPAN // P              # 4 chunks per span

DT_PROJ = F32R
DT_SC = F32R
DT_AV = F32R


def _build(nc, tc, dt_proj, dt_sc, dt_av):
    # host pre-packs both inputs into SBUF layout (plain 2D DMAs, max-size
    # contiguous DRAM runs): xt[p, (ns, kt, c)], wc[p, (kt, m)] with
    # m = packed weight columns [q01 | k01 | v01 | k2+v2 | q2]
    xt = nc.dram_tensor("xt", [P, N * KT], dt_proj, kind="ExternalInput").ap()
    wc = nc.dram_tensor("wc", [P, KT * 3 * DL], dt_proj,
                        kind="ExternalInput").ap()
    o = nc.dram_tensor("o", [HL * (HD + 1), N], F32, kind="ExternalOutput").ap()

    with ExitStack() as ctx:
        pool = lambda name, bufs, **kw: ctx.enter_context(
            tc.tile_pool(name=name, bufs=bufs, **kw))
        const_pool = pool("const", 1)
        xpool = pool("x", NS)  # 2 tags x NS halves
        wpool = pool("w", 3)
        qk_pool = pool("qk", NS)      # qT01, qT2z, (vT01, vT2z share size)
        kz_pool = pool("kz", HL * NS)
        vnat_pool = pool("vnat", HL)
        ppool = pool("p", 4)
        osb_pool = pool("osb", 3)
        ps_proj = pool("ps_proj", 2, space="PSUM")
        ps_sc = pool("ps_sc", 2, space="PSUM")   # [128,1024] pair tiles: 2 banks each
        ps_av = pool("ps_av", 2, space="PSUM")

        ident = const_pool.tile([P, P], F32)
        make_identity(nc, ident[:])
        identr = const_pool.tile([P, P], F32R)
        nc.vector.tensor_copy(identr[:], ident[:])
        trimask = const_pool.tile([P, P], F32)
        make_upper_triangular(nc, trimask[:], val=1.0, diag=True)
        ones32 = const_pool.tile([P, 2 * NT], F32)
        nc.gpsimd.memset(ones32[:], 1.0)
        zeros = const_pool.tile([P, SPAN], F32)
        nc.gpsimd.memset(zeros[:], 0.0)

        warm = ps_proj.tile([P, P], F32, tag="ps_proj", name="warm")
        for _ in range(26):
            nc.tensor.matmul(warm[:], ident[:], ident[:], start=True, stop=True)

        def zfill(ap):
            """Zero a region of a possibly-f32r tile (memset can't write f32r)."""
            if ap.dtype == F32:
                nc.gpsimd.memset(ap, 0.0)
            else:
                nc.vector.tensor_copy(ap, zeros[0:ap.shape[0], 0:ap.shape[1]])

        # ---- DMA inputs (one batched 3D-AP DMA per tensor / span).
        # x span 0 goes first on the HWDGE queue; weights ride the gpsimd
        # SWDGE queue in parallel so the first projection starts ASAP.
        KH = KT // 2
        x_tiles = [xpool.tile([P, KT * SPAN], dt_proj, tag="x", name=f"x{i}")
                   for i in range(NS)]

        def x_slice(ns, kt):
            return x_tiles[ns][:, kt * SPAN:(kt + 1) * SPAN]

        def dma_x(ns):
            w = KT * SPAN
            nc.sync.dma_start(x_tiles[ns][:], xt[:, ns * w:(ns + 1) * w])

        WCW = 3 * DL
        w_half = [wpool.tile([P, KH * WCW], dt_proj, tag=f"w{i}", name=f"w{i}")
                  for i in range(2)]

        def w_slice(kt, c0, c1):
            t = w_half[kt // KH]
            b = (kt % KH) * WCW
            return t[:, b + c0:b + c1]

        def dma_w(half):
            w = KH * WCW
            nc.sync.dma_start(w_half[half][:], wc[:, half * w:(half + 1) * w])

        dma_w(0)
        dma_x(0)
        dma_w(1)
        for ns in range(1, NS):
            dma_x(ns)

        # ---- padded q/k/v layout (zero-filled up front, off critical path) ----
        qT01 = [qk_pool.tile([P, SPAN], dt_sc, tag="q01", name=f"q01_{i}") for i in range(NS)]
        qT2z = [qk_pool.tile([P, SPAN], dt_sc, tag="q2z", name=f"q2z_{i}") for i in range(NS)]
        vT01 = [qk_pool.tile([P, SPAN], dt_av, tag="v01", name=f"v01_{i}") for i in range(NS)]
        vT2z = [qk_pool.tile([P, SPAN], dt_av, tag="v2z", name=f"v2z_{i}") for i in range(NS)]
        kTz = [[kz_pool.tile([P, SPAN], dt_sc, tag="kz", name=f"kz_{h}_{i}")
                for i in range(NS)] for h in range(HL)]
        for ns in range(NS):
            zfill(qT2z[ns][HD:P, :])
            zfill(vT2z[ns][0:HD, :])
            zfill(kTz[0][ns][HD:P, :])
            zfill(kTz[1][ns][0:HD, :])
            zfill(kTz[2][ns][HD:P, :])

        # v in natural layout: heads 0,1 interleaved per j-tile as
        # [v0(64) | 1 | v1(64) | 1]; head 2 separate as [v2(64) | 1].
        v_nat01 = vnat_pool.tile([P, NT * 2 * (HD + 1)], dt_av, tag="vnat01")
        v_nat2 = vnat_pool.tile([P, NT * (HD + 1)], dt_av, tag="vnat2")
        c01 = v_nat01[:].rearrange("p (t c) -> p t c", c=HD + 1)[:, :, HD]
        c2 = v_nat2[:].rearrange("p (t c) -> p t c", c=HD + 1)[:, :, HD]
        if dt_av == F32:
            nc.gpsimd.memset(c01, 1.0)
            nc.gpsimd.memset(c2, 1.0)
        else:
            nc.vector.tensor_copy(c01, ones32[:])
            nc.vector.tensor_copy(c2, ones32[:, 0:NT])

        def vnat_lhsT(h, jt):
            if h < 2:
                b = jt * 2 * (HD + 1) + h * (HD + 1)
                return v_nat01[:, b:b + HD + 1]
            b = jt * (HD + 1)
            return v_nat2[:, b:b + HD + 1]

        # ---- projections: 5 M-chunks per span (q01, k01, v01, k2+v2, q2) ----
        m_chunks = ((0, P, "q01"), (P, P, "k01"), (2 * P, P, "v01"),
                    (3 * P, P, "k2v2"), (4 * P, HD, "q2"))
        for ns in range(NS):
            for (moff, msz, what) in m_chunks:
                pt = ps_proj.tile([msz, SPAN], F32, tag="ps_proj")
                for kt in range(KT):
                    nc.tensor.matmul(
                        pt[:],
                        w_slice(kt, moff, moff + msz),
                        x_slice(ns, kt),
                        start=(kt == 0), stop=(kt == KT - 1))
                if what == "q01":
                    nc.vector.tensor_copy(qT01[ns][:], pt[:])
                elif what == "k01":
                    nc.vector.tensor_copy(kTz[0][ns][0:HD, :], pt[0:HD, :])
                    nc.vector.tensor_copy(kTz[1][ns][HD:P, :], pt[HD:P, :])
                elif what == "v01":
                    nc.vector.tensor_copy(vT01[ns][:], pt[:])
                elif what == "k2v2":
                    nc.vector.tensor_copy(kTz[2][ns][0:HD, :], pt[0:HD, :])
                    nc.vector.tensor_copy(vT2z[ns][HD:P, :], pt[HD:P, :])
                else:
                    nc.vector.tensor_copy(qT2z[ns][0:HD, :], pt[:])

            # v -> natural layout for this span's 4 j-tiles (K=128 transposes)
            for c in range(CPS):
                jt = ns * CPS + c
                tp = ps_proj.tile([P, P], dt_av, tag="ps_proj")
                nc.tensor.transpose(tp[:], vT01[ns][:, c * P:(c + 1) * P],
                                    identr[:] if dt_av == F32R else ident[:])
                b01 = jt * 2 * (HD + 1)
                nc.vector.tensor_copy(
                    v_nat01[:].rearrange("p (t c) -> p t c", c=HD + 1)[
                        :, 2 * jt:2 * jt + 2, 0:HD],
                    tp[:].rearrange("p (t c) -> p t c", c=HD))
                tp2 = ps_proj.tile([P, P], dt_av, tag="ps_proj")
                nc.tensor.transpose(tp2[:], vT2z[ns][:, c * P:(c + 1) * P],
                                    identr[:] if dt_av == F32R else ident[:])
                nc.vector.tensor_copy(
                    v_nat2[:, jt * (HD + 1):jt * (HD + 1) + HD], tp2[:, HD:P])

            # ---- attention: span s == ns only needs projections <= ns ----
            s = ns
            njt = CPS * s + CPS

            def finalize(h, av):
                ob = osb_pool.tile([HD + 1, SPAN], F32, tag="osb")
                nc.vector.tensor_copy(ob[:], av[:])
                nc.sync.dma_start(
                    o[h * (HD + 1):(h + 1) * (HD + 1),
                      s * SPAN:(s + 1) * SPAN], ob[:])

            # heads 0 and 1 fused: one 2-bank score psum, one exp, two AVs
            av0 = ps_av.tile([HD + 1, SPAN], F32, tag="ps_av", name=f"av0_{s}")
            av1 = ps_av.tile([HD + 1, SPAN], F32, tag="ps_av", name=f"av1_{s}")
            for jt in range(njt):
                c_d = jt - CPS * s
                n0 = max(c_d, 0) * P
                ns_k, ck = jt // CPS, jt % CPS
                w = SPAN - n0
                sc = ps_sc.tile([P, 2 * SPAN], F32, tag="ps_sc")
                nc.tensor.matmul(
                    sc[:, n0:SPAN],
                    kTz[0][ns_k][:, ck * P:(ck + 1) * P],
                    qT01[s][:, n0:SPAN], start=True, stop=True)
                nc.tensor.matmul(
                    sc[:, SPAN + n0:2 * SPAN],
                    kTz[1][ns_k][:, ck * P:(ck + 1) * P],
                    qT01[s][:, n0:SPAN], start=True, stop=True)
                p = ppool.tile([P, 2 * SPAN], dt_av, tag="p")
                sc3 = sc[:].rearrange("q (t c) -> q t c", c=SPAN)
                p3 = p[:].rearrange("q (t c) -> q t c", c=SPAN)
                nc.scalar.activation(p3[:, :, n0:SPAN], sc3[:, :, n0:SPAN],
                                     mybir.ActivationFunctionType.Exp)
                if c_d >= 0:
                    nc.vector.tensor_mul(
                        p[:, n0:n0 + P], p[:, n0:n0 + P], trimask[:])
                    nc.vector.tensor_mul(
                        p[:, SPAN + n0:SPAN + n0 + P],
                        p[:, SPAN + n0:SPAN + n0 + P], trimask[:])
                nc.tensor.matmul(
                    av0[:, n0:SPAN], vnat_lhsT(0, jt), p[:, n0:SPAN],
                    start=(jt == 0), stop=(jt == njt - 1))
                nc.tensor.matmul(
                    av1[:, n0:SPAN], vnat_lhsT(1, jt),
                    p[:, SPAN + n0:2 * SPAN],
                    start=(jt == 0), stop=(jt == njt - 1))
            finalize(0, av0)
            finalize(1, av1)

            # head 2 solo (uses first half of a pair-sized score tile)
            av2 = ps_av.tile([HD + 1, SPAN], F32, tag="ps_av", name=f"av2_{s}")
            for jt in range(njt):
                c_d = jt - CPS * s
                n0 = max(c_d, 0) * P
                ns_k, ck = jt // CPS, jt % CPS
                sc = ps_sc.tile([P, 2 * SPAN], F32, tag="ps_sc")
                nc.tensor.matmul(
                    sc[:, n0:SPAN],
                    kTz[2][ns_k][:, ck * P:(ck + 1) * P],
                    qT2z[s][:, n0:SPAN], start=True, stop=True)
                p = ppool.tile([P, 2 * SPAN], dt_av, tag="p")
                nc.scalar.activation(p[:, n0:SPAN], sc[:, n0:SPAN],
                                     mybir.ActivationFunctionType.Exp)
                if c_d >= 0:
                    nc.vector.tensor_mul(
                        p[:, n0:n0 + P], p[:, n0:n0 + P], trimask[:])
                nc.tensor.matmul(
                    av2[:, n0:SPAN], vnat_lhsT(2, jt), p[:, n0:SPAN],
                    start=(jt == 0), stop=(jt == njt - 1))
            finalize(2, av2)


_NC_CACHE = {}


def _get_module(dt_proj=DT_PROJ, dt_sc=DT_SC, dt_av=DT_AV):
    key = (dt_proj, dt_sc, dt_av)
    if key not in _NC_CACHE:
        nc = bacc.Bacc("TRN2", target_bir_lowering=False, debug=False)
        with tile.TileContext(nc) as tc:
            _build(nc, tc, dt_proj, dt_sc, dt_av)
        nc.compile()
        _NC_CACHE[key] = nc
    return _NC_CACHE[key]


def _in_maps(x, Wq, Wk, Wv):
    maps = []
    xT = [np.ascontiguousarray(
        x[b].T.reshape(KT, P, NS, SPAN).transpose(1, 2, 0, 3).reshape(P, -1))
        for b in range(B)]
    WqT, WkT, WvT = Wq.T, Wk.T, Wv.T
    for c in range(N_CORES):
        bc, g = divmod(c, N_CORES // B)
        s0 = g * DL
        wcomb = np.concatenate([
            WqT[:, s0:s0 + P], WkT[:, s0:s0 + P], WvT[:, s0:s0 + P],
            WkT[:, s0 + P:s0 + DL], WvT[:, s0 + P:s0 + DL],
            WqT[:, s0 + P:s0 + DL]], axis=1)
        wpk = np.ascontiguousarray(
            wcomb.reshape(KT, P, 3 * DL).transpose(1, 0, 2).reshape(P, -1))
        maps.append({
            "xt": xT[bc],
            "wc": wpk,
        })
    return maps


def kernel(x, Wq, Wk, Wv, _trace=False, _tmpdir=None, **_kw):
    x = np.asarray(x, dtype=np.float32)
    Wq = np.asarray(Wq, dtype=np.float32)
    Wk = np.asarray(Wk, dtype=np.float32)
    Wv = np.asarray(Wv, dtype=np.float32)
    assert x.shape == (B, N, D) and Wq.shape == (D, D)

    nc = _get_module()
    res = bass_utils.run_bass_kernel_spmd(
        nc, _in_maps(x, Wq, Wk, Wv), core_ids=list(range(N_CORES)),
        trace=_trace, tmpdir=_tmpdir)
    out = np.empty((B, N, D), np.float32)
    for c in range(N_CORES):
        bc, g = divmod(c, N_CORES // B)
        oT = res.results[c]["o"].astype(np.float64)
        for h in range(HL):
            blk = oT[h * (HD + 1):h * (HD + 1) + HD, :]
            den = oT[h * (HD + 1) + HD, :]
            out[bc, :, g * DL + h * HD:g * DL + (h + 1) * HD] = \
                (blk / den).T.astype(np.float32)
    if _trace:
        return out, res
    return out


# revision 24
# speedup vs baseline: 1.1526x; 1.1526x over previous
"""Causal multi-head attention (b=2, n=2048, d=768, 12 heads) on 8 TRN2 NeuronCores.

Sharding: batch x head-group. Core c handles batch c//4 and heads 3*(c%4) .. 3*(c%4)+2.
Each core gets xT = x[b].T plus W.T column slices for its 3 heads, computes the
unnormalized attention output (transposed) plus softmax denominators; the host
divides, transposes, and concatenates slabs into the full [2, 2048, 768].

Per-core algorithm (everything transposed so softmax reductions ride on matmuls):
  qT/kT/vT = (W.T slice).T @ xT            TensorE, per 512-col span
  v_nat[j, m] = transpose(vT) + ones column -> stationary [128, 65] per j-tile
  per head, per 512-col i-span:
    sT[j, i] = kT_h[:, jtile].T @ qT[:, span]   (psum, causally skipped/sliced)
    p = exp(sT)  unshifted (max causal score ~66 fits fp32); diagonal 128-blocks
        multiplied by a 0/1 triangular mask
    av[0:65, span] += v_nat[jtile].T @ p    (row 64 accumulates sum(p) = denom)
  av -> DRAM; host computes (av[0:64]/av[64]).T per head.

Perf-critical TRN2 facts baked in (measured on hardware):
  - every matmul keeps contraction K=128 (zero-padded kT/qT2): K<128 streams leave
    the PE HAM clock gate at 1.2 GHz and stop back-to-back pipelining (~3x slower)
  - float32r inputs: 1.06 cyc/row pipelined, ~12-bit mantissa
  - f32r tiles can only be produced by DVE/ACT compute ops (not memset/HWDGE-DMA),
    so zero-fills of f32r tiles are DVE copies from an f32 zeros tile
"""
import sys

if "/opt/trn_rl_repo" not in sys.path:
    sys.path.insert(0, "/opt/trn_rl_repo")

from contextlib import ExitStack

import numpy as np

import concourse.bass as bass
import concourse.tile as tile
from concourse import bacc, mybir, bass_utils
from concourse.masks import make_identity, make_upper_triangular

F32 = mybir.dt.float32
F32R = mybir.dt.float32r

P = 128
SPAN = 512
HD = 64

B, N, D, NH = 2, 2048, 768, 12
HL = 3                       # heads per core
DL = HL * HD                 # 192
N_CORES = 8
KT = D // P                  # 6 contraction chunks
NS = N // SPAN               # 4 spans
NT = N // P                  # 16 j-tiles
CPS = SPAN // P              # 4 chunks per span

DT_PROJ = F32R
DT_SC = F32R
DT_AV = F32R


def _build(nc, tc, dt_proj, dt_sc, dt_av):
    # host pre-packs both inputs into SBUF layout (plain 2D DMAs, max-size
    # contiguous DRAM runs): xt[p, (ns, kt, c)], wc[p, (kt, m)] with
    # m = packed weight columns [q01 | k01 | v01 | k2+v2 | q2]
    xt = nc.dram_tensor("xt", [P, N * KT], dt_proj, kind="ExternalInput").ap()
    wc = nc.dram_tensor("wc", [P, KT * 3 * DL], dt_proj,
                        kind="ExternalInput").ap()
    o = nc.dram_tensor("o", [HL * (HD + 1), N], F32, kind="ExternalOutput").ap()

    with ExitStack() as ctx:
        pool = lambda name, bufs, **kw: ctx.enter_context(
            tc.tile_pool(name=name, bufs=bufs, **kw))
        const_pool = pool("const", 1)
        xpool = pool("x", NS)  # 2 tags x NS halves
        wpool = pool("w", 3)
        qk_pool = pool("qk", NS)      # qT01, qT2z, (vT01, vT2z share size)
        kz_pool = pool("kz", HL * NS)
        vnat_pool = pool("vnat", HL)
        ppool = pool("p", 3)
        osb_pool = pool("osb", 3)
        ps_proj = pool("ps_proj", 2, space="PSUM")
        ps_sc = pool("ps_sc", 2, space="PSUM")   # [128,1024] pair tiles: 2 banks each
        ps_av = pool("ps_av", 2, space="PSUM")

        ident = const_pool.tile([P, P], F32)
        make_identity(nc, ident[:])
        trimask = const_pool.tile([P, P], F32)
        make_upper_triangular(nc, trimask[:], val=1.0, diag=True)
        ones32 = const_pool.tile([P, 2 * NT], F32)
        nc.gpsimd.memset(ones32[:], 1.0)
        zeros = const_pool.tile([P, SPAN], F32)
        nc.gpsimd.memset(zeros[:], 0.0)

        warm = ps_proj.tile([P, P], F32, tag="ps_proj", name="warm")
        for _ in range(26):
            nc.tensor.matmul(warm[:], ident[:], ident[:], start=True, stop=True)

        def zfill(ap):
            """Zero a region of a possibly-f32r tile (memset can't write f32r)."""
            if ap.dtype == F32:
                nc.gpsimd.memset(ap, 0.0)
            else:
                nc.vector.tensor_copy(ap, zeros[0:ap.shape[0], 0:ap.shape[1]])

        # ---- DMA inputs (one batched 3D-AP DMA per tensor / span).
        # x span 0 goes first on the HWDGE queue; weights ride the gpsimd
        # SWDGE queue in parallel so the first projection starts ASAP.
        KH = KT // 2
        x_tiles = [xpool.tile([P, KT * SPAN], dt_proj, tag="x", name=f"x{i}")
                   for i in range(NS)]

        def x_slice(ns, kt):
            return x_tiles[ns][:, kt * SPAN:(kt + 1) * SPAN]

        def dma_x(ns):
            w = KT * SPAN
            nc.sync.dma_start(x_tiles[ns][:], xt[:, ns * w:(ns + 1) * w])

        WCW = 3 * DL
        w_half = [wpool.tile([P, KH * WCW], dt_proj, tag=f"w{i}", name=f"w{i}")
                  for i in range(2)]

        def w_slice(kt, c0, c1):
            t = w_half[kt // KH]
            b = (kt % KH) * WCW
            return t[:, b + c0:b + c1]

        def dma_w(half):
            w = KH * WCW
            nc.sync.dma_start(w_half[half][:], wc[:, half * w:(half + 1) * w])

        dma_w(0)
        dma_x(0)
        dma_w(1)
        for ns in range(1, NS):
            dma_x(ns)

        # ---- padded q/k/v layout (zero-filled up front, off critical path) ----
        qT01 = [qk_pool.tile([P, SPAN], dt_sc, tag="q01", name=f"q01_{i}") for i in range(NS)]
        qT2z = [qk_pool.tile([P, SPAN], dt_sc, tag="q2z", name=f"q2z_{i}") for i in range(NS)]
        vT01 = [qk_pool.tile([P, SPAN], F32, tag="v01", name=f"v01_{i}") for i in range(NS)]
        vT2z = [qk_pool.tile([P, SPAN], F32, tag="v2z", name=f"v2z_{i}") for i in range(NS)]
        kTz = [[kz_pool.tile([P, SPAN], dt_sc, tag="kz", name=f"kz_{h}_{i}")
                for i in range(NS)] for h in range(HL)]
        for ns in range(NS):
            zfill(qT2z[ns][HD:P, :])
            zfill(vT2z[ns][0:HD, :])
            zfill(kTz[0][ns][HD:P, :])
            zfill(kTz[1][ns][0:HD, :])
            zfill(kTz[2][ns][HD:P, :])

        # v in natural layout: heads 0,1 interleaved per j-tile as
        # [v0(64) | 1 | v1(64) | 1]; head 2 separate as [v2(64) | 1].
        v_nat01 = vnat_pool.tile([P, NT * 2 * (HD + 1)], dt_av, tag="vnat01")
        v_nat2 = vnat_pool.tile([P, NT * (HD + 1)], dt_av, tag="vnat2")
        c01 = v_nat01[:].rearrange("p (t c) -> p t c", c=HD + 1)[:, :, HD]
        c2 = v_nat2[:].rearrange("p (t c) -> p t c", c=HD + 1)[:, :, HD]
        if dt_av == F32:
            nc.gpsimd.memset(c01, 1.0)
            nc.gpsimd.memset(c2, 1.0)
        else:
            nc.vector.tensor_copy(c01, ones32[:])
            nc.vector.tensor_copy(c2, ones32[:, 0:NT])

        def vnat_lhsT(h, jt):
            if h < 2:
                b = jt * 2 * (HD + 1) + h * (HD + 1)
                return v_nat01[:, b:b + HD + 1]
            b = jt * (HD + 1)
            return v_nat2[:, b:b + HD + 1]

        # ---- projections: 5 M-chunks per span (q01, k01, v01, k2+v2, q2) ----
        m_chunks = ((0, P, "q01"), (P, P, "k01"), (2 * P, P, "v01"),
                    (3 * P, P, "k2v2"), (4 * P, HD, "q2"))
        for ns in range(NS):
            for (moff, msz, what) in m_chunks:
                pt = ps_proj.tile([msz, SPAN], F32, tag="ps_proj")
                for kt in range(KT):
                    nc.tensor.matmul(
                        pt[:],
                        w_slice(kt, moff, moff + msz),
                        x_slice(ns, kt),
                        start=(kt == 0), stop=(kt == KT - 1))
                if what == "q01":
                    nc.vector.tensor_copy(qT01[ns][:], pt[:])
                elif what == "k01":
                    nc.vector.tensor_copy(kTz[0][ns][0:HD, :], pt[0:HD, :])
                    nc.vector.tensor_copy(kTz[1][ns][HD:P, :], pt[HD:P, :])
                elif what == "v01":
                    nc.vector.tensor_copy(vT01[ns][:], pt[:])
                elif what == "k2v2":
                    nc.vector.tensor_copy(kTz[2][ns][0:HD, :], pt[0:HD, :])
                    nc.vector.tensor_copy(vT2z[ns][HD:P, :], pt[HD:P, :])
                else:
                    nc.vector.tensor_copy(qT2z[ns][0:HD, :], pt[:])

            # v -> natural layout for this span's 4 j-tiles (K=128 transposes)
            for c in range(CPS):
                jt = ns * CPS + c
                tp = ps_proj.tile([P, P], F32, tag="ps_proj")
                nc.tensor.transpose(tp[:], vT01[ns][:, c * P:(c + 1) * P],
                                    ident[:])
                b01 = jt * 2 * (HD + 1)
                nc.vector.tensor_copy(
                    v_nat01[:].rearrange("p (t c) -> p t c", c=HD + 1)[
                        :, 2 * jt:2 * jt + 2, 0:HD],
                    tp[:].rearrange("p (t c) -> p t c", c=HD))
                tp2 = ps_proj.tile([P, P], F32, tag="ps_proj")
                nc.tensor.transpose(tp2[:], vT2z[ns][:, c * P:(c + 1) * P],
                                    ident[:])
                nc.vector.tensor_copy(
                    v_nat2[:, jt * (HD + 1):jt * (HD + 1) + HD], tp2[:, HD:P])

            # ---- attention: span s == ns only needs projections <= ns ----
            s = ns
            njt = CPS * s + CPS

            def finalize(h, av):
                ob = osb_pool.tile([HD + 1, SPAN], F32, tag="osb")
                nc.vector.tensor_copy(ob[:], av[:])
                nc.sync.dma_start(
                    o[h * (HD + 1):(h + 1) * (HD + 1),
                      s * SPAN:(s + 1) * SPAN], ob[:])

            # heads 0 and 1 fused: one 2-bank score psum, one exp, two AVs
            av0 = ps_av.tile([HD + 1, SPAN], F32, tag="ps_av", name=f"av0_{s}")
            av1 = ps_av.tile([HD + 1, SPAN], F32, tag="ps_av", name=f"av1_{s}")
            for jt in range(njt):
                c_d = jt - CPS * s
                n0 = max(c_d, 0) * P
                ns_k, ck = jt // CPS, jt % CPS
                w = SPAN - n0
                sc = ps_sc.tile([P, 2 * SPAN], F32, tag="ps_sc")
                nc.tensor.matmul(
                    sc[:, n0:SPAN],
                    kTz[0][ns_k][:, ck * P:(ck + 1) * P],
                    qT01[s][:, n0:SPAN], start=True, stop=True)
                nc.tensor.matmul(
                    sc[:, SPAN + n0:2 * SPAN],
                    kTz[1][ns_k][:, ck * P:(ck + 1) * P],
                    qT01[s][:, n0:SPAN], start=True, stop=True)
                p = ppool.tile([P, 2 * SPAN], dt_av, tag="p")
                sc3 = sc[:].rearrange("q (t c) -> q t c", c=SPAN)
                p3 = p[:].rearrange("q (t c) -> q t c", c=SPAN)
                nc.scalar.activation(p3[:, :, n0:SPAN], sc3[:, :, n0:SPAN],
                                     mybir.ActivationFunctionType.Exp)
                if c_d >= 0:
                    nc.vector.tensor_mul(
                        p[:, n0:n0 + P], p[:, n0:n0 + P], trimask[:])
                    nc.vector.tensor_mul(
                        p[:, SPAN + n0:SPAN + n0 + P],
                        p[:, SPAN + n0:SPAN + n0 + P], trimask[:])
                nc.tensor.matmul(
                    av0[:, n0:SPAN], vnat_lhsT(0, jt), p[:, n0:SPAN],
                    start=(jt == 0), stop=(jt == njt - 1))
                nc.tensor.matmul(
                    av1[:, n0:SPAN], vnat_lhsT(1, jt),
                    p[:, SPAN + n0:2 * SPAN],
                    start=(jt == 0), stop=(jt == njt - 1))
            finalize(0, av0)
            finalize(1, av1)

            # head 2 solo (uses first half of a pair-sized score tile)
            av2 = ps_av.tile([HD + 1, SPAN], F32, tag="ps_av", name=f"av2_{s}")
            for jt in range(njt):
                c_d = jt - CPS * s
                n0 = max(c_d, 0) * P
                ns_k, ck = jt // CPS, jt % CPS
                sc = ps_sc.tile([P, 2 * SPAN], F32, tag="ps_sc")
                nc.tensor.matmul(
                    sc[:, n0:SPAN],
                    kTz[2][ns_k][:, ck * P:(ck + 1) * P],
                    qT2z[s][:, n0:SPAN], start=True, stop=True)
                p = ppool.tile([P, 2 * SPAN], dt_av, tag="p")
                nc.scalar.activation(p[:, n0:SPAN], sc[:, n0:SPAN],
                                     mybir.ActivationFunctionType.Exp)
                if c_d >= 0:
                    nc.vector.tensor_mul(
                        p[:, n0:n0 + P], p[:, n0:n0 + P], trimask[:])
                nc.tensor.matmul(
                    av2[:, n0:SPAN], vnat_lhsT(2, jt), p[:, n0:SPAN],
                    start=(jt == 0), stop=(jt == njt - 1))
            finalize(2, av2)


_NC_CACHE = {}


def _get_module(dt_proj=DT_PROJ, dt_sc=DT_SC, dt_av=DT_AV):
    key = (dt_proj, dt_sc, dt_av)
    if key not in _NC_CACHE:
        nc = bacc.Bacc("TRN2", target_bir_lowering=False, debug=False)
        with tile.TileContext(nc) as tc:
            _build(nc, tc, dt_proj, dt_sc, dt_av)
        nc.compile()
        _NC_CACHE[key] = nc
    return _NC_CACHE[key]


def _in_maps(x, Wq, Wk, Wv):
    maps = []
    xT = [np.ascontiguousarray(
        x[b].T.reshape(KT, P, NS, SPAN).transpose(1, 2, 0, 3).reshape(P, -1))
        for b in range(B)]
    WqT, WkT, WvT = Wq.T, Wk.T, Wv.T
    for c in range(N_CORES):
        bc, g = divmod(c, N_CORES // B)
        s0 = g * DL
        wcomb = np.concatenate([
            WqT[:, s0:s0 + P], WkT[:, s0:s0 + P], WvT[:, s0:s0 + P],
            WkT[:, s0 + P:s0 + DL], WvT[:, s0 + P:s0 + DL],
            WqT[:, s0 + P:s0 + DL]], axis=1)
        wpk = np.ascontiguousarray(
            wcomb.reshape(KT, P, 3 * DL).transpose(1, 0, 2).reshape(P, -1))
        maps.append({
            "xt": xT[bc],
            "wc": wpk,
        })
    return maps


def kernel(x, Wq, Wk, Wv, _trace=False, _tmpdir=None, **_kw):
    x = np.asarray(x, dtype=np.float32)
    Wq = np.asarray(Wq, dtype=np.float32)
    Wk = np.asarray(Wk, dtype=np.float32)
    Wv = np.asarray(Wv, dtype=np.float32)
    assert x.shape == (B, N, D) and Wq.shape == (D, D)

    nc = _get_module()
    res = bass_utils.run_bass_kernel_spmd(
        nc, _in_maps(x, Wq, Wk, Wv), core_ids=list(range(N_CORES)),
        trace=_trace, tmpdir=_tmpdir)
    out = np.empty((B, N, D), np.float32)
    for c in range(N_CORES):
        bc, g = divmod(c, N_CORES // B)
        oT = res.results[c]["o"].astype(np.float64)
        for h in range(HL):
            blk = oT[h * (HD + 1):h * (HD + 1) + HD, :]
            den = oT[h * (HD + 1) + HD, :]
            out[bc, :, g * DL + h * HD:g * DL + (h + 1) * HD] = \
                (blk / den).T.astype(np.float32)
    if _trace:
        return out, res
    return out
